# revision 1
# baseline (speedup 1.0000x reference)
"""GatedConv GNN message passing on 8 TRN2 NeuronCores.

Strategy:
- Nodes sharded contiguously across 8 cores (6250/core, padded to 6272=49*128).
- Edges sharded by dst owner, sorted by dst, grouped into 128-node dst blocks,
  padded to a uniform tiles-per-block capacity so one SPMD program serves all
  cores.
- Per layer: AllGather h (bf16) -> per 128-edge tile: indirect-DMA row gather
  of h_full[src] + host-precomputed one-hot dst mask -> PE matmul
  (h_g.T @ mask) accumulated in PSUM per dst block = transposed segment sum.
  Conv weight is folded AFTER aggregation (linearity). GRU runs in transposed
  [feature, node] layout; PE transposes produce the row-major h for the next
  AllGather / final pooling.
- Mean-pool via host-built batch one-hot matmul + 1/count scale; host sums the
  8 per-core partials (unshard-reduce).
"""
import contextlib
import ctypes
import os
import sys
import types

import numpy as np

from concourse import bass, mybir, tile
from concourse.bass_utils import run_bass_kernel_spmd

NCORES = 8
P = 128
D = 128
G = 64
N = 50000
V = 100000
NUM_LAYERS = 2
NL = N // NCORES            # 6250 nodes per core
NB = (NL + P - 1) // P      # 49 dst blocks per core
NLP = NB * P                # 6272 padded nodes per core
NFULL = NCORES * NLP        # 50176 rows in allgathered h

_F32 = mybir.dt.float32
_BF16 = mybir.dt.bfloat16
_I32 = mybir.dt.int32


# ---------------------------------------------------------------- wait split
def _split_waits(nc):
    """walrus allows only ONE sync-wait per instruction; hoist extras onto
    NoOps just before, on the same engine stream (sequencer order)."""
    uid = 0
    n_fixed = 0
    for bb in nc.main_func.blocks:
        out = []
        for ins in bb.instructions:
            si = getattr(ins, "sync_info", None)
            if si is not None and len(si.on_wait) > 1:
                for w in si.on_wait[:-1]:
                    uid += 1
                    out.append(mybir.InstNoOp(
                        name=f"WSPLIT-{uid}", engine=ins.engine,
                        bass_nofuse=True, ins=[], outs=[],
                        sync_info=mybir.SyncInfo(on_wait=[w], on_update=[]),
                    ))
                ins.sync_info = mybir.SyncInfo(
                    on_wait=[si.on_wait[-1]], on_update=si.on_update)
                n_fixed += 1
            out.append(ins)
        bb.instructions = out
    return n_fixed


# ---------------------------------------------------------------- ntff hook
def _install_ntff_hook():
    import antenv
    if "antenv.axon_hooks" in sys.modules:
        return
    mod = types.ModuleType("antenv.axon_hooks")
    _state = {"hook": None}
    mod.set_axon_ntff_profile_hook = lambda h: _state.__setitem__("hook", h)
    mod.get_axon_ntff_profile_hook = lambda: _state["hook"]
    sys.modules["antenv.axon_hooks"] = mod
    antenv.axon_hooks = mod
    if "/root/.axon_site" not in sys.path:
        sys.path.insert(0, "/root/.axon_site")
    try:
        from trn_agent_boot.trn_boot import _ntff_profile_via_ctypes
        hook = _ntff_profile_via_ctypes("/opt/axon/libaxon_pjrt.so")
        mod.set_axon_ntff_profile_hook(hook)
    except Exception:
        pass


# ---------------------------------------------------------------- builder
def _build(cap: int, phases: int = 99):
    """cap = max edge tiles per (core, dst-block); uniform across cores."""
    nc = bass.Bass(num_devices=NCORES)
    T = NB * cap  # edge tiles per core per layer

    embed_in = nc.declare_dram_parameter("embed", [V, D], _F32, isOutput=False)
    nid_in = nc.declare_dram_parameter("nid", [P, NB], _I32, isOutput=False)
    src_in = nc.declare_dram_parameter("srcidx", [P, T], _I32, isOutput=False)
    mask_in = nc.declare_dram_parameter("masks", [T * P, D], _BF16, isOutput=False)
    pool_in = nc.declare_dram_parameter("pool1h", [P, NB * G], _BF16, isOutput=False)
    cinv_in = nc.declare_dram_parameter("cinv", [G, 1], _F32, isOutput=False)
    convw_in = nc.declare_dram_parameter("convw", [D, NUM_LAYERS * D], _F32, isOutput=False)
    wih_in = nc.declare_dram_parameter("wihT", [D, 3 * D], _F32, isOutput=False)
    whh_in = nc.declare_dram_parameter("whhT", [D, 3 * D], _F32, isOutput=False)
    bias_in = nc.declare_dram_parameter("biases", [P, 4], _F32, isOutput=False)
    out_ext = nc.declare_dram_parameter("out", [G, D], _F32, isOutput=True)

    ag_in = [nc.dram_tensor(f"ag_in{l}", [NLP, D], _BF16) for l in range(NUM_LAYERS)]
    ag_out = [nc.dram_tensor(f"ag_out{l}", [NFULL, D], _BF16, addr_space="Shared")
              for l in range(NUM_LAYERS)]

    with tile.TileContext(nc) as tc:
        with contextlib.ExitStack() as stk:
            const = stk.enter_context(tc.tile_pool(name="const", bufs=1))
            sb = stk.enter_context(tc.tile_pool(name="sb", bufs=3))
            pp = stk.enter_context(tc.tile_pool(name="pp", bufs=2, space="PSUM"))
            gpsum = stk.enter_context(tc.tile_pool(name="gpsum", bufs=1, space="PSUM"))

            # ---- constants / weights ----
            src_sb = const.tile([P, T], _I32)
            nc.sync.dma_start(out=src_sb[:], in_=src_in[:])
            nid_sb = const.tile([P, NB], _I32)
            nc.sync.dma_start(out=nid_sb[:], in_=nid_in[:])
            pool_sb = const.tile([P, NB * G], _BF16)
            nc.sync.dma_start(out=pool_sb[:], in_=pool_in[:])
            cinv_sb = const.tile([G, 1], _F32)
            nc.sync.dma_start(out=cinv_sb[:], in_=cinv_in[:])
            bias_sb = const.tile([P, 4], _F32)
            nc.sync.dma_start(out=bias_sb[:], in_=bias_in[:])

            def _load_bf16(src_ap, shape, nm):
                t32 = sb.tile(shape, _F32, name=f"t32_{nm}", tag=f"t32_{nm}")
                nc.sync.dma_start(out=t32[:], in_=src_ap)
                tb = const.tile(shape, _BF16, name=f"bf_{nm}", tag=f"bf_{nm}")
                nc.scalar.copy(out=tb[:], in_=t32[:])
                return tb

            convw_sb = _load_bf16(convw_in[:], [D, NUM_LAYERS * D], "convw")
            wih_sb = _load_bf16(wih_in[:], [D, 3 * D], "wih")
            whh_sb = _load_bf16(whh_in[:], [D, 3 * D], "whh")

            from concourse.masks import make_identity
            ident = const.tile([P, P], _BF16)
            make_identity(nc, ident[:])

            # ---- persistent state buffers ----
            hT = [const.tile([P, NLP], _BF16, name=f"hT{i}", tag=f"hT{i}") for i in range(2)]
            hnorm = const.tile([P, NLP], _BF16)   # [node-part, d] per 128-block, col-block b
            aggT = const.tile([P, NLP], _BF16)

            # ---- phase 1: embed gather -> hnorm + hT[0] ----
            for b in range(NB):
                g32 = sb.tile([P, D], _F32, tag="embg")
                nc.gpsimd.indirect_dma_start(
                    out=g32[:], out_offset=None, in_=embed_in[:],
                    in_offset=bass.IndirectOffsetOnAxis(ap=nid_sb[:, b:b + 1], axis=0))
                nc.scalar.copy(out=hnorm[:, b * D:(b + 1) * D], in_=g32[:])
                tp = pp.tile([P, P], _BF16, tag="scratch", space="PSUM")
                nc.tensor.transpose(out=tp[:], in_=hnorm[:, b * D:(b + 1) * D], identity=ident[:])
                nc.scalar.copy(out=hT[0][:, b * P:(b + 1) * P], in_=tp[:])
            nc.sync.dma_start(
                out=ag_in[0][:].rearrange("(b p) d -> p b d", p=P),
                in_=hnorm[:].rearrange("p (b d) -> p b d", d=D))

            # ---- layers ----
            for l in range(NUM_LAYERS if phases >= 2 else 0):
                nc.gpsimd.collective_compute(
                    "AllGather", mybir.AluOpType.bypass,
                    replica_groups=[list(range(NCORES))],
                    ins=[ag_in[l][:]], outs=[ag_out[l][:]])

                # edge phase: per dst block, segment-sum via mask matmuls in PSUM
                for b in range(NB):
                    pagg = pp.tile([P, P], _F32, tag="scratch", space="PSUM")
                    mblk = sb.tile([P, cap * D], _BF16, tag="mblk")
                    nc.sync.dma_start(
                        out=mblk[:].rearrange("p (t d) -> p t d", d=D),
                        in_=mask_in[b * cap * P:(b + 1) * cap * P, :].rearrange(
                            "(t p) d -> p t d", p=P))
                    for t in range(cap):
                        tt = b * cap + t
                        gt = sb.tile([P, D], _BF16, tag="gath")
                        nc.gpsimd.indirect_dma_start(
                            out=gt[:], out_offset=None, in_=ag_out[l][:],
                            in_offset=bass.IndirectOffsetOnAxis(ap=src_sb[:, tt:tt + 1], axis=0))
                        nc.tensor.matmul(out=pagg[:], lhsT=gt[:], rhs=mblk[:, t * D:(t + 1) * D],
                                         start=(t == 0), stop=(t == cap - 1))
                    nc.scalar.copy(out=aggT[:, b * P:(b + 1) * P], in_=pagg[:])

                if phases < 3:
                    continue
                # conv + GRU phase, slabs of 512 nodes
                W = 512
                nslab = NLP // W if NLP % W == 0 else NLP // W + 1
                hT_next = hT[(l + 1) % 2]
                for s in range(nslab):
                    c0 = s * W
                    w = min(W, NLP - c0)
                    cs = slice(c0, c0 + w)
                    xt_ps = gpsum.tile([P, W], _F32, tag="gi0", space="PSUM")
                    nc.tensor.matmul(out=xt_ps[:, :w], lhsT=convw_sb[:, l * D:(l + 1) * D],
                                     rhs=aggT[:, cs], start=True, stop=True)
                    xt_sb = sb.tile([P, W], _BF16, tag="xtsb")
                    nc.scalar.copy(out=xt_sb[:, :w], in_=xt_ps[:, :w])

                    gi = []
                    gh = []
                    for gidx in range(3):
                        gps = gpsum.tile([P, W], _F32, tag=f"gi{gidx}", space="PSUM")
                        nc.tensor.matmul(out=gps[:, :w], lhsT=wih_sb[:, gidx * D:(gidx + 1) * D],
                                         rhs=xt_sb[:, :w], start=True, stop=True)
                        gi.append(gps)
                        hps = gpsum.tile([P, W], _F32, tag=f"gh{gidx}", space="PSUM")
                        nc.tensor.matmul(out=hps[:, :w], lhsT=whh_sb[:, gidx * D:(gidx + 1) * D],
                                         rhs=hT[l % 2][:, cs], start=True, stop=True)
                        gh.append(hps)

                    # r = sigmoid(gi_r + gh_r + b_r) ; z likewise
                    r_sb = sb.tile([P, W], _F32, tag="r")
                    nc.scalar.activation(out=r_sb[:, :w], in_=gh[0][:, :w],
                                         func=mybir.ActivationFunctionType.Identity,
                                         bias=bias_sb[:, 0:1])
                    nc.vector.tensor_tensor(out=r_sb[:, :w], in0=gi[0][:, :w], in1=r_sb[:, :w],
                                            op=mybir.AluOpType.add)
                    nc.scalar.activation(out=r_sb[:, :w], in_=r_sb[:, :w],
                                         func=mybir.ActivationFunctionType.Sigmoid)
                    z_sb = sb.tile([P, W], _F32, tag="z")
                    nc.scalar.activation(out=z_sb[:, :w], in_=gh[1][:, :w],
                                         func=mybir.ActivationFunctionType.Identity,
                                         bias=bias_sb[:, 1:2])
                    nc.vector.tensor_tensor(out=z_sb[:, :w], in0=gi[1][:, :w], in1=z_sb[:, :w],
                                            op=mybir.AluOpType.add)
                    nc.scalar.activation(out=z_sb[:, :w], in_=z_sb[:, :w],
                                         func=mybir.ActivationFunctionType.Sigmoid)
                    # n = tanh(gi_n + b_in + r * (gh_n + b_hn))
                    hn_sb = sb.tile([P, W], _F32, tag="hn")
                    nc.scalar.activation(out=hn_sb[:, :w], in_=gh[2][:, :w],
                                         func=mybir.ActivationFunctionType.Identity,
                                         bias=bias_sb[:, 3:4])
                    nc.vector.tensor_tensor(out=hn_sb[:, :w], in0=r_sb[:, :w], in1=hn_sb[:, :w],
                                            op=mybir.AluOpType.mult)
                    nc.vector.tensor_tensor(out=hn_sb[:, :w], in0=hn_sb[:, :w], in1=gi[2][:, :w],
                                            op=mybir.AluOpType.add)
                    nc.scalar.activation(out=hn_sb[:, :w], in_=hn_sb[:, :w],
                                         func=mybir.ActivationFunctionType.Tanh,
                                         bias=bias_sb[:, 2:3])
                    # h' = n + z*(h - n)
                    d_sb = sb.tile([P, W], _F32, tag="d")
                    nc.vector.tensor_tensor(out=d_sb[:, :w], in0=hT[l % 2][:, cs], in1=hn_sb[:, :w],
                                            op=mybir.AluOpType.subtract)
                    nc.vector.tensor_tensor(out=d_sb[:, :w], in0=z_sb[:, :w], in1=d_sb[:, :w],
                                            op=mybir.AluOpType.mult)
                    nc.vector.tensor_tensor(out=hT_next[:, cs], in0=d_sb[:, :w], in1=hn_sb[:, :w],
                                            op=mybir.AluOpType.add)

                # transpose h'T back to row-major hnorm
                for b in range(NB):
                    tp = pp.tile([P, P], _BF16, tag="scratch", space="PSUM")
                    nc.tensor.transpose(out=tp[:], in_=hT_next[:, b * P:(b + 1) * P],
                                        identity=ident[:])
                    nc.scalar.copy(out=hnorm[:, b * D:(b + 1) * D], in_=tp[:])
                if l + 1 < NUM_LAYERS:
                    nc.sync.dma_start(
                        out=ag_in[l + 1][:].rearrange("(b p) d -> p b d", p=P),
                        in_=hnorm[:].rearrange("p (b d) -> p b d", d=D))

            # ---- pool ----
            if phases < 4:
                out_sb0 = sb.tile([G, D], _F32, tag="outsb")
                nc.vector.memset(out_sb0[:], 0.0)
                nc.sync.dma_start(out=out_ext[:], in_=out_sb0[:])
            else:
                ppool = pp.tile([G, D], _F32, tag="scratch", space="PSUM")
                for b in range(NB):
                    nc.tensor.matmul(out=ppool[:], lhsT=pool_sb[:, b * G:(b + 1) * G],
                                     rhs=hnorm[:, b * D:(b + 1) * D],
                                     start=(b == 0), stop=(b == NB - 1))
                out_sb = sb.tile([G, D], _F32, tag="outsb")
                nc.vector.tensor_scalar(out=out_sb[:], in0=ppool[:], scalar1=cinv_sb[:, 0:1],
                                        scalar2=None, op0=mybir.AluOpType.mult)
                nc.sync.dma_start(out=out_ext[:], in_=out_sb[:])

    _split_waits(nc)
    return nc


_CACHE = {}


def kernel(node_ids, edge_index, batch, num_graphs, embed, conv_w, w_ih, w_hh,
           b_ih, b_hh) -> np.ndarray:
    import ml_dtypes
    bf16 = ml_dtypes.bfloat16

    node_ids = np.asarray(node_ids)
    edge_index = np.asarray(edge_index)
    batch = np.asarray(batch)
    embed = np.asarray(embed, dtype=np.float32)
    conv_w = np.asarray(conv_w, dtype=np.float32)
    w_ih = np.asarray(w_ih, dtype=np.float32)
    w_hh = np.asarray(w_hh, dtype=np.float32)
    b_ih = np.asarray(b_ih, dtype=np.float32)
    b_hh = np.asarray(b_hh, dtype=np.float32)
    G_ = int(num_graphs)
    assert G_ == G and node_ids.shape[0] == N

    src_all = edge_index[0].astype(np.int64)
    dst_all = edge_index[1].astype(np.int64)

    # shard edges by dst owner; per (core, block) group edges; uniform capacity
    owner = dst_all // NL
    per_core = []
    max_tiles = 1
    for c in range(NCORES):
        sel = owner == c
        src_c = src_all[sel]
        dst_c = dst_all[sel] - c * NL          # 0..NL-1
        blk = dst_c // P
        rel = dst_c % P
        order = np.argsort(blk * P + rel, kind="stable")
        src_c, blk, rel = src_c[order], blk[order], rel[order]
        counts = np.bincount(blk, minlength=NB)
        max_tiles = max(max_tiles, int(np.ceil(counts.max() / P)))
        per_core.append((src_c, blk, rel, counts))
    cap = max_tiles
    T = NB * cap

    # global padded row index of node n in ag_out
    def padded_idx(n):
        return (n // NL) * NLP + (n % NL)

    in_maps = []
    # common tensors
    convw_arr = np.ascontiguousarray(np.concatenate([conv_w[i] for i in range(NUM_LAYERS)], axis=1))
    wihT = np.ascontiguousarray(w_ih.T)           # [128, 384]
    whhT = np.ascontiguousarray(w_hh.T)
    biases = np.zeros((P, 4), np.float32)
    biases[:, 0] = b_ih[0:D] + b_hh[0:D]          # r
    biases[:, 1] = b_ih[D:2 * D] + b_hh[D:2 * D]  # z
    biases[:, 2] = b_ih[2 * D:3 * D]              # in
    biases[:, 3] = b_hh[2 * D:3 * D]              # hn
    counts_g = np.bincount(batch, minlength=G).astype(np.float32)
    cinv = (1.0 / np.maximum(counts_g, 1.0)).reshape(G, 1).astype(np.float32)

    eye = np.eye(P, dtype=bf16)
    for c in range(NCORES):
        src_c, blk, rel, counts = per_core[c]
        srcidx = np.zeros((P, T), np.int32)
        masks = np.zeros((T * P, D), dtype=bf16)
        pos = 0
        for b in range(NB):
            nb_e = int(counts[b])
            e_src = padded_idx(src_c[pos:pos + nb_e]).astype(np.int32)
            e_rel = rel[pos:pos + nb_e].astype(np.int64)
            pos += nb_e
            for t in range(cap):
                tt = b * cap + t
                lo = t * P
                sl_src = e_src[lo:lo + P]
                sl_rel = e_rel[lo:lo + P]
                k = sl_src.shape[0]
                if k:
                    srcidx[:k, tt] = sl_src
                    masks[tt * P:tt * P + k, :] = eye[sl_rel]
        # node ids per padded slot, [128, NB] column-major tiles
        nid = np.zeros((P, NB), np.int32)
        ids_c = node_ids[c * NL:(c + 1) * NL].astype(np.int32)
        ids_pad = np.zeros(NLP, np.int32)
        ids_pad[:NL] = ids_c
        nid[:, :] = ids_pad.reshape(NB, P).T
        # pool one-hot [128, NB*G]
        b_c = batch[c * NL:(c + 1) * NL].astype(np.int64)
        p1h = np.zeros((NLP, G), dtype=bf16)
        p1h[np.arange(NL), b_c] = np.float32(1.0)
        pool1h = np.zeros((P, NB * G), dtype=bf16)
        for b in range(NB):
            pool1h[:, b * G:(b + 1) * G] = p1h[b * P:(b + 1) * P, :]

        in_maps.append({
            "embed": embed, "nid": nid, "srcidx": srcidx, "masks": masks,
            "pool1h": pool1h, "cinv": cinv, "convw": convw_arr,
            "wihT": wihT, "whhT": whhT, "biases": biases,
        })

    if cap not in _CACHE:
        _CACHE[cap] = _build(cap)
    nc = _CACHE[cap]

    trace = bool(int(os.environ.get("BASS_GNN_TRACE", "0")))
    if trace:
        _install_ntff_hook()
    res = run_bass_kernel_spmd(nc, in_maps, core_ids=list(range(NCORES)),
                               trace=trace)
    if trace:
        kernel.last_exec_time_ns = res.exec_time_ns
        kernel.last_results = res
    outs = [r["out"] for r in res.results]
    return np.sum(np.stack(outs, 0), axis=0, dtype=np.float32)


kernel.last_exec_time_ns = None



# revision 13
# speedup vs baseline: 1.1738x; 1.1738x over previous
"""GatedConv GNN message passing on 8 TRN2 NeuronCores.

Strategy:
- Nodes sharded contiguously across 8 cores (6250/core, padded to 6272=49*128).
- Edges sharded by dst owner, sorted by dst, grouped into 128-node dst blocks.
  Per (core, block) the edges are split into "low" (padded src idx < 32768)
  and "high" ranges so indices fit int16, padded to uniform per-block tile
  capacities CAPL/CAPH so one SPMD program serves all cores.
- Per layer: AllGather h (bf16) -> per group of GB dst blocks: two batched
  dma_gather calls (InstDMAGatherAnt; ~1us SWDGE fixed cost amortized over
  ~5-10k rows) fetch h_full[src] rows; host-precomputed one-hot dst masks
  stream from DRAM; PE matmul (h_g.T @ mask) accumulates in PSUM per dst
  block = transposed segment sum. Conv weight folded AFTER aggregation
  (linearity). GRU runs in transposed [feature, node] layout; PE transposes
  produce the row-major h for the next AllGather / final pooling.
- Mean-pool via host-built batch one-hot matmul + 1/count scale; host sums
  the 8 per-core partials (unshard-reduce).
"""
import contextlib
import os
import sys
import types

import numpy as np

from concourse import bass, mybir, tile
from concourse.bass_utils import run_bass_kernel_spmd

NCORES = 8
P = 128
D = 128
G = 64
N = 50000
V = 100000
NUM_LAYERS = 2
NL = N // NCORES            # 6250 nodes per core
NB = (NL + P - 1) // P      # 49 dst blocks per core
NLP = NB * P                # 6272 padded nodes per core
NFULL = NCORES * NLP        # 50176 rows in allgathered h
LOWN = 32768                # int16 index range split point
GB = 1                      # dst blocks per gather group (SWDGE ring capacity
                            # bounds per-call descriptor count)

_F32 = mybir.dt.float32
_BF16 = mybir.dt.bfloat16
_I32 = mybir.dt.int32
_I16 = mybir.dt.int16


# ---------------------------------------------------------------- wait split
def _split_waits(nc):
    """walrus allows only ONE sync-wait per instruction; hoist extras onto
    NoOps just before, on the same engine stream (sequencer order)."""
    uid = 0
    n_fixed = 0
    for bb in nc.main_func.blocks:
        out = []
        for ins in bb.instructions:
            si = getattr(ins, "sync_info", None)
            if si is not None and len(si.on_wait) > 1:
                for w in si.on_wait[:-1]:
                    uid += 1
                    out.append(mybir.InstNoOp(
                        name=f"WSPLIT-{uid}", engine=ins.engine,
                        bass_nofuse=True, ins=[], outs=[],
                        sync_info=mybir.SyncInfo(on_wait=[w], on_update=[]),
                    ))
                ins.sync_info = mybir.SyncInfo(
                    on_wait=[si.on_wait[-1]], on_update=si.on_update)
                n_fixed += 1
            out.append(ins)
        bb.instructions = out
    return n_fixed


# ------------------------------------------------------------- library loads
def _insert_library_loads(nc):
    """Bacc compile steps the plain-Bass path skips: insert GPSIMD library
    reloads for custom instructions (dma_gather needs the mlp library) and
    encode the resulting pseudo instructions into real MPC ISA structs."""
    import bass_rust as _bass_rust
    from concourse.library_config import all_libraries, standard
    inst_type_to_lib_mask = {}
    for lib in all_libraries:
        for inst_type in lib.instructions:
            inst_type_to_lib_mask[inst_type] = inst_type_to_lib_mask.get(
                inst_type, 0) | (1 << lib.index)
    _bass_rust.insert_library_loads(
        nc, inst_type_to_lib_mask, len(all_libraries), standard.index)
    mybir.codegen_inst_isa_subclasses(nc)


# ---------------------------------------------------------------- ntff hook
def _install_ntff_hook():
    import antenv
    if "antenv.axon_hooks" in sys.modules:
        return
    mod = types.ModuleType("antenv.axon_hooks")
    _state = {"hook": None}
    mod.set_axon_ntff_profile_hook = lambda h: _state.__setitem__("hook", h)
    mod.get_axon_ntff_profile_hook = lambda: _state["hook"]
    sys.modules["antenv.axon_hooks"] = mod
    antenv.axon_hooks = mod
    if "/root/.axon_site" not in sys.path:
        sys.path.insert(0, "/root/.axon_site")
    try:
        from trn_agent_boot.trn_boot import _ntff_profile_via_ctypes
        hook = _ntff_profile_via_ctypes("/opt/axon/libaxon_pjrt.so")
        mod.set_axon_ntff_profile_hook(hook)
    except Exception:
        pass


def _groups():
    """Block groups for gather batching: sizes [GB]*q + [r]."""
    out = []
    j0 = 0
    while j0 < NB:
        gb = min(GB, NB - j0)
        out.append((j0, gb))
        j0 += gb
    return out


# ---------------------------------------------------------------- builder
MAXC = 8   # max tiles (128 idxs each) per dma_gather call: the fixed
           # 1024-descriptor SWDGE ring hangs on larger calls (HW-probed)


def _build(capl: int, caph: int, phases: int = 99):
    nc = bass.Bass(num_devices=NCORES)
    tpb = capl + caph           # tiles per dst block
    T = NB * tpb                # mask tiles per core per layer
    groups = _groups()
    # idx16 column offsets per (group, range)
    colw_l = [gb * capl * 8 for _, gb in groups]
    colw_h = [gb * caph * 8 for _, gb in groups]
    cols_total = sum(colw_l) + sum(colw_h)

    embed_in = nc.declare_dram_parameter("embed", [V, D], _F32, isOutput=False)
    nid_in = nc.declare_dram_parameter("nid", [P, NB], _I32, isOutput=False)
    idx_in = nc.declare_dram_parameter("idx16", [P, cols_total], _I16, isOutput=False)
    mask_in = nc.declare_dram_parameter("masks", [P, T * D], _BF16, isOutput=False)
    pool_in = nc.declare_dram_parameter("pool1h", [P, NB * G], _BF16, isOutput=False)
    cinv_in = nc.declare_dram_parameter("cinv", [G, 1], _F32, isOutput=False)
    convw_in = nc.declare_dram_parameter("convw", [D, NUM_LAYERS * D], _F32, isOutput=False)
    wih_in = nc.declare_dram_parameter("wihT", [D, 3 * D], _F32, isOutput=False)
    whh_in = nc.declare_dram_parameter("whhT", [D, 3 * D], _F32, isOutput=False)
    bias_in = nc.declare_dram_parameter("biases", [P, 4], _F32, isOutput=False)
    out_ext = nc.declare_dram_parameter("out", [G, D], _F32, isOutput=True)

    ag_in = [nc.dram_tensor(f"ag_in{l}", [NLP, D], _F32) for l in range(NUM_LAYERS)]
    ag_out = [nc.dram_tensor(f"ag_out{l}", [NFULL, D], _F32, addr_space="Shared")
              for l in range(NUM_LAYERS)]

    with tile.TileContext(nc) as tc:
        with contextlib.ExitStack() as stk:
            const = stk.enter_context(tc.tile_pool(name="const", bufs=1))
            sb = stk.enter_context(tc.tile_pool(name="sb", bufs=3))
            gp = stk.enter_context(tc.tile_pool(name="gp", bufs=2))
            pp = stk.enter_context(tc.tile_pool(name="pp", bufs=2, space="PSUM"))
            gpsum = stk.enter_context(tc.tile_pool(name="gpsum", bufs=1, space="PSUM"))

            # ---- constants / weights ----
            idx_sb = const.tile([P, cols_total], _I16)
            nc.sync.dma_start(out=idx_sb[:], in_=idx_in[:])
            nid_sb = const.tile([P, NB], _I32)
            nc.sync.dma_start(out=nid_sb[:], in_=nid_in[:])
            pool_sb = const.tile([P, NB * G], _BF16)
            nc.sync.dma_start(out=pool_sb[:], in_=pool_in[:])
            cinv_sb = const.tile([G, 1], _F32)
            nc.sync.dma_start(out=cinv_sb[:], in_=cinv_in[:])
            bias_sb = const.tile([P, 4], _F32)
            nc.sync.dma_start(out=bias_sb[:], in_=bias_in[:])

            def _load_bf16(src_ap, shape, nm):
                t32 = sb.tile(shape, _F32, name=f"t32_{nm}", tag=f"t32_{nm}")
                nc.sync.dma_start(out=t32[:], in_=src_ap)
                tb = const.tile(shape, _BF16, name=f"bf_{nm}", tag=f"bf_{nm}")
                nc.scalar.copy(out=tb[:], in_=t32[:])
                return tb

            convw_sb = _load_bf16(convw_in[:], [D, NUM_LAYERS * D], "convw")
            wih_sb = _load_bf16(wih_in[:], [D, 3 * D], "wih")
            whh_sb = _load_bf16(whh_in[:], [D, 3 * D], "whh")

            from concourse.masks import make_identity
            ident = const.tile([P, P], _BF16)
            make_identity(nc, ident[:])

            # ---- persistent state buffers ----
            hT = [const.tile([P, NLP], _BF16, name=f"hT{i}", tag=f"hT{i}") for i in range(2)]
            hnorm = const.tile([P, NLP], _BF16)   # row-major h: block b at cols [b*D,(b+1)*D)
            aggT = const.tile([P, NLP], _BF16)

            # ---- phase 1: embed gather -> hnorm + hT[0] ----
            for b in range(NB):
                g32 = sb.tile([P, D], _F32, tag="embg")
                nc.gpsimd.indirect_dma_start(
                    out=g32[:], out_offset=None, in_=embed_in[:],
                    in_offset=bass.IndirectOffsetOnAxis(ap=nid_sb[:, b:b + 1], axis=0))
                nc.scalar.copy(out=hnorm[:, b * D:(b + 1) * D], in_=g32[:])
                tp = pp.tile([P, P], _BF16, tag="scratch", space="PSUM")
                nc.tensor.transpose(out=tp[:], in_=hnorm[:, b * D:(b + 1) * D], identity=ident[:])
                nc.scalar.copy(out=hT[0][:, b * P:(b + 1) * P], in_=tp[:])
            nc.gpsimd.dma_start(
                out=ag_in[0][:].rearrange("(b p) d -> p b d", p=P),
                in_=hnorm[:].rearrange("p (b d) -> p b d", d=D))

            # num_idxs register per distinct value (to_reg per call exhausts
            # the gpsimd register pool)
            nidx_regs = {}
            for cap in (capl, caph):
                a = 0
                while a < cap:
                    k = min(MAXC, cap - a)
                    if k * P not in nidx_regs:
                        nidx_regs[k * P] = nc.gpsimd.to_reg(k * P)
                    a += k

            # ---- layers ----
            for l in range(NUM_LAYERS if phases >= 2 else 0):
                nc.gpsimd.collective_compute(
                    "AllGather", mybir.AluOpType.bypass,
                    replica_groups=[list(range(NCORES))],
                    ins=[ag_in[l][:]], outs=[ag_out[l][:]])

                # edge phase: batched row gathers + per-block mask matmuls
                colp = 0
                for gi, (j0, gb) in enumerate(groups):
                    g32 = gp.tile([P, gb * tpb * D], _F32, tag="g32")
                    for base, cap, src_ap in (
                            (0, capl, ag_out[l][0:LOWN, :]),
                            (capl, caph, ag_out[l][LOWN:NFULL, :])):
                        a = 0
                        while a < cap:
                            k = min(MAXC, cap - a)
                            nc.gpsimd.dma_gather(
                                out_ap=g32[:, (base + a) * D:(base + a + k) * D]
                                    .rearrange("p (t d) -> p t d", d=D),
                                in_ap=src_ap,
                                idxs_ap=idx_sb[:, colp + a * 8:colp + (a + k) * 8],
                                num_idxs=k * P, num_idxs_reg=nidx_regs[k * P],
                                elem_size=D)
                            a += k
                        colp += cap * 8
                    gblk = gp.tile([P, gb * tpb * D], _BF16, tag="gath")
                    nc.vector.tensor_copy(out=gblk[:], in_=g32[:])

                    for j in range(gb):
                        b = j0 + j
                        pagg = pp.tile([P, P], _F32, tag="scratch", space="PSUM")
                        mblk = sb.tile([P, tpb * D], _BF16, tag="mblk")
                        nc.sync.dma_start(
                            out=mblk[:], in_=mask_in[:, b * tpb * D:(b + 1) * tpb * D])
                        for t in range(tpb):
                            if t < capl:
                                chunk = j * capl + t
                            else:
                                chunk = gb * capl + j * caph + (t - capl)
                            nc.tensor.matmul(
                                out=pagg[:], lhsT=gblk[:, chunk * D:(chunk + 1) * D],
                                rhs=mblk[:, t * D:(t + 1) * D],
                                start=(t == 0), stop=(t == tpb - 1))
                        nc.scalar.copy(out=aggT[:, b * P:(b + 1) * P], in_=pagg[:])

                if phases < 3:
                    continue
                # conv + GRU phase, slabs of 512 nodes
                W = 512
                nslab = NLP // W if NLP % W == 0 else NLP // W + 1
                hT_next = hT[(l + 1) % 2]
                for s in range(nslab):
                    c0 = s * W
                    w = min(W, NLP - c0)
                    cs = slice(c0, c0 + w)
                    xt_ps = gpsum.tile([P, W], _F32, tag="gi0", space="PSUM")
                    nc.tensor.matmul(out=xt_ps[:, :w], lhsT=convw_sb[:, l * D:(l + 1) * D],
                                     rhs=aggT[:, cs], start=True, stop=True)
                    xt_sb = sb.tile([P, W], _BF16, tag="xtsb")
                    nc.scalar.copy(out=xt_sb[:, :w], in_=xt_ps[:, :w])

                    gi = []
                    gh = []
                    for gidx in range(3):
                        gps = gpsum.tile([P, W], _F32, tag=f"gi{gidx}", space="PSUM")
                        nc.tensor.matmul(out=gps[:, :w], lhsT=wih_sb[:, gidx * D:(gidx + 1) * D],
                                         rhs=xt_sb[:, :w], start=True, stop=True)
                        gi.append(gps)
                        hps = gpsum.tile([P, W], _F32, tag=f"gh{gidx}", space="PSUM")
                        nc.tensor.matmul(out=hps[:, :w], lhsT=whh_sb[:, gidx * D:(gidx + 1) * D],
                                         rhs=hT[l % 2][:, cs], start=True, stop=True)
                        gh.append(hps)

                    # r = sigmoid(gi_r + gh_r + b_r) ; z likewise
                    r_sb = sb.tile([P, W], _F32, tag="r")
                    nc.scalar.activation(out=r_sb[:, :w], in_=gh[0][:, :w],
                                         func=mybir.ActivationFunctionType.Identity,
                                         bias=bias_sb[:, 0:1])
                    nc.vector.tensor_tensor(out=r_sb[:, :w], in0=gi[0][:, :w], in1=r_sb[:, :w],
                                            op=mybir.AluOpType.add)
                    nc.scalar.activation(out=r_sb[:, :w], in_=r_sb[:, :w],
                                         func=mybir.ActivationFunctionType.Sigmoid)
                    z_sb = sb.tile([P, W], _F32, tag="z")
                    nc.scalar.activation(out=z_sb[:, :w], in_=gh[1][:, :w],
                                         func=mybir.ActivationFunctionType.Identity,
                                         bias=bias_sb[:, 1:2])
                    nc.vector.tensor_tensor(out=z_sb[:, :w], in0=gi[1][:, :w], in1=z_sb[:, :w],
                                            op=mybir.AluOpType.add)
                    nc.scalar.activation(out=z_sb[:, :w], in_=z_sb[:, :w],
                                         func=mybir.ActivationFunctionType.Sigmoid)
                    # n = tanh(gi_n + b_in + r * (gh_n + b_hn))
                    hn_sb = sb.tile([P, W], _F32, tag="hn")
                    nc.scalar.activation(out=hn_sb[:, :w], in_=gh[2][:, :w],
                                         func=mybir.ActivationFunctionType.Identity,
                                         bias=bias_sb[:, 3:4])
                    nc.vector.tensor_tensor(out=hn_sb[:, :w], in0=r_sb[:, :w], in1=hn_sb[:, :w],
                                            op=mybir.AluOpType.mult)
                    nc.vector.tensor_tensor(out=hn_sb[:, :w], in0=hn_sb[:, :w], in1=gi[2][:, :w],
                                            op=mybir.AluOpType.add)
                    nc.scalar.activation(out=hn_sb[:, :w], in_=hn_sb[:, :w],
                                         func=mybir.ActivationFunctionType.Tanh,
                                         bias=bias_sb[:, 2:3])
                    # h' = n + z*(h - n)
                    d_sb = sb.tile([P, W], _F32, tag="d")
                    nc.vector.tensor_tensor(out=d_sb[:, :w], in0=hT[l % 2][:, cs], in1=hn_sb[:, :w],
                                            op=mybir.AluOpType.subtract)
                    nc.vector.tensor_tensor(out=d_sb[:, :w], in0=z_sb[:, :w], in1=d_sb[:, :w],
                                            op=mybir.AluOpType.mult)
                    nc.vector.tensor_tensor(out=hT_next[:, cs], in0=d_sb[:, :w], in1=hn_sb[:, :w],
                                            op=mybir.AluOpType.add)

                # transpose h'T back to row-major hnorm
                for b in range(NB):
                    tp = pp.tile([P, P], _BF16, tag="scratch", space="PSUM")
                    nc.tensor.transpose(out=tp[:], in_=hT_next[:, b * P:(b + 1) * P],
                                        identity=ident[:])
                    nc.scalar.copy(out=hnorm[:, b * D:(b + 1) * D], in_=tp[:])
                if l + 1 < NUM_LAYERS:
                    nc.gpsimd.dma_start(
                        out=ag_in[l + 1][:].rearrange("(b p) d -> p b d", p=P),
                        in_=hnorm[:].rearrange("p (b d) -> p b d", d=D))

            # ---- pool ----
            if phases < 4:
                out_sb0 = sb.tile([G, D], _F32, tag="outsb")
                nc.vector.memset(out_sb0[:], 0.0)
                nc.sync.dma_start(out=out_ext[:], in_=out_sb0[:])
            else:
                ppool = pp.tile([G, D], _F32, tag="scratch", space="PSUM")
                for b in range(NB):
                    nc.tensor.matmul(out=ppool[:], lhsT=pool_sb[:, b * G:(b + 1) * G],
                                     rhs=hnorm[:, b * D:(b + 1) * D],
                                     start=(b == 0), stop=(b == NB - 1))
                out_sb = sb.tile([G, D], _F32, tag="outsb")
                nc.vector.tensor_scalar(out=out_sb[:], in0=ppool[:], scalar1=cinv_sb[:, 0:1],
                                        scalar2=None, op0=mybir.AluOpType.mult)
                nc.sync.dma_start(out=out_ext[:], in_=out_sb[:])

    _split_waits(nc)
    _insert_library_loads(nc)
    return nc


_CACHE = {}


def kernel(node_ids, edge_index, batch, num_graphs, embed, conv_w, w_ih, w_hh,
           b_ih, b_hh) -> np.ndarray:
    import ml_dtypes
    bf16 = ml_dtypes.bfloat16

    node_ids = np.asarray(node_ids)
    edge_index = np.asarray(edge_index)
    batch = np.asarray(batch)
    embed = np.asarray(embed, dtype=np.float32)
    conv_w = np.asarray(conv_w, dtype=np.float32)
    w_ih = np.asarray(w_ih, dtype=np.float32)
    w_hh = np.asarray(w_hh, dtype=np.float32)
    b_ih = np.asarray(b_ih, dtype=np.float32)
    b_hh = np.asarray(b_hh, dtype=np.float32)
    G_ = int(num_graphs)
    assert G_ == G and node_ids.shape[0] == N

    src_all = edge_index[0].astype(np.int64)
    dst_all = edge_index[1].astype(np.int64)

    # global padded row index of node n in ag_out
    def padded_idx(n):
        return (n // NL) * NLP + (n % NL)

    # ---- pass 1: per-core per-block low/high edge lists; find capacities ----
    owner = dst_all // NL
    per_core = []
    capl = caph = 1
    for c in range(NCORES):
        sel = owner == c
        src_c = padded_idx(src_all[sel]).astype(np.int64)
        dst_c = dst_all[sel] - c * NL          # 0..NL-1
        blk = dst_c // P
        rel = dst_c % P
        lows = []
        highs = []
        for b in range(NB):
            bs = blk == b
            s_b, r_b = src_c[bs], rel[bs]
            lo = s_b < LOWN
            lows.append((s_b[lo], r_b[lo]))
            highs.append((s_b[~lo] - LOWN, r_b[~lo]))
            capl = max(capl, -(-lows[-1][0].shape[0] // P))
            caph = max(caph, -(-highs[-1][0].shape[0] // P))
        per_core.append((lows, highs))
    tpb = capl + caph
    T = NB * tpb
    groups = _groups()

    # ---- common tensors ----
    convw_arr = np.ascontiguousarray(np.concatenate([conv_w[i] for i in range(NUM_LAYERS)], axis=1))
    wihT = np.ascontiguousarray(w_ih.T)           # [128, 384]
    whhT = np.ascontiguousarray(w_hh.T)
    biases = np.zeros((P, 4), np.float32)
    biases[:, 0] = b_ih[0:D] + b_hh[0:D]          # r
    biases[:, 1] = b_ih[D:2 * D] + b_hh[D:2 * D]  # z
    biases[:, 2] = b_ih[2 * D:3 * D]              # in
    biases[:, 3] = b_hh[2 * D:3 * D]              # hn
    counts_g = np.bincount(batch, minlength=G).astype(np.float32)
    cinv = (1.0 / np.maximum(counts_g, 1.0)).reshape(G, 1).astype(np.float32)

    # idx16 columns per group/range
    cols_total = sum(gb * capl * 8 + gb * caph * 8 for _, gb in groups)

    in_maps = []
    for c in range(NCORES):
        lows, highs = per_core[c]
        masks = np.zeros((P, T * D), dtype=bf16)
        idx16 = np.zeros((16, cols_total), dtype=np.int16)
        colp = 0
        for j0, gb in groups:
            for rng, cap, base in ((0, capl, 0), (1, caph, capl)):
                nidx = gb * cap * P
                seq = np.zeros(nidx, np.int16)
                for j in range(gb):
                    b = j0 + j
                    s_b, r_b = (lows[b] if rng == 0 else highs[b])
                    k = s_b.shape[0]
                    seq[j * cap * P:j * cap * P + k] = s_b.astype(np.int16)
                    # mask entries: edge i -> tile i//P, slot i%P
                    ar = np.arange(k)
                    tt = b * tpb + base + ar // P
                    masks[ar % P, tt * D + r_b] = np.float32(1.0)
                idx16[:, colp:colp + nidx // 16] = seq.reshape(-1, 16).T
                colp += nidx // 16
        idx16_full = np.tile(idx16, (8, 1))       # replicate across Q7 cores

        # node ids per padded slot, [128, NB] column-major tiles
        nid = np.zeros((P, NB), np.int32)
        ids_c = node_ids[c * NL:(c + 1) * NL].astype(np.int32)
        ids_pad = np.zeros(NLP, np.int32)
        ids_pad[:NL] = ids_c
        nid[:, :] = ids_pad.reshape(NB, P).T
        # pool one-hot [128, NB*G]
        b_c = batch[c * NL:(c + 1) * NL].astype(np.int64)
        p1h = np.zeros((NLP, G), dtype=bf16)
        p1h[np.arange(NL), b_c] = np.float32(1.0)
        pool1h = np.zeros((P, NB * G), dtype=bf16)
        for b in range(NB):
            pool1h[:, b * G:(b + 1) * G] = p1h[b * P:(b + 1) * P, :]

        in_maps.append({
            "embed": embed, "nid": nid, "idx16": idx16_full, "masks": masks,
            "pool1h": pool1h, "cinv": cinv, "convw": convw_arr,
            "wihT": wihT, "whhT": whhT, "biases": biases,
        })

    key = (capl, caph)
    if key not in _CACHE:
        _CACHE[key] = _build(capl, caph)
    nc = _CACHE[key]

    if bool(int(os.environ.get("BASS_GNN_SIM", "0"))):
        from concourse.bass_interp import MultiCoreSim
        sim = MultiCoreSim(nc, num_cores=NCORES)
        for c in range(NCORES):
            for nm, arr in in_maps[c].items():
                sim.cores[c].tensor(nm)[:] = arr
        sim.simulate(check_with_hw=False)
        outs = [np.array(sim.cores[c].tensor("out")) for c in range(NCORES)]
        return np.sum(np.stack(outs, 0), axis=0, dtype=np.float32)

    trace = bool(int(os.environ.get("BASS_GNN_TRACE", "0")))
    if trace:
        _install_ntff_hook()
    res = run_bass_kernel_spmd(nc, in_maps, core_ids=list(range(NCORES)),
                               trace=trace)
    if trace:
        kernel.last_exec_time_ns = res.exec_time_ns
        kernel.last_results = res
    outs = [r["out"] for r in res.results]
    return np.sum(np.stack(outs, 0), axis=0, dtype=np.float32)


kernel.last_exec_time_ns = None


# revision 15
# speedup vs baseline: 1.5432x; 1.3148x over previous
"""GatedConv GNN message passing on 8 TRN2 NeuronCores.

Strategy:
- Nodes sharded contiguously across 8 cores (6250/core, padded to 6272=49*128).
- Edges sharded by dst owner, sorted by dst, grouped into 128-node dst blocks.
  Per (core, block) the edges are split into "low" (padded src idx < 32768)
  and "high" ranges so indices fit int16, padded to uniform per-block tile
  capacities CAPL/CAPH so one SPMD program serves all cores.
- Per layer: AllGather h (bf16) -> per group of GB dst blocks: two batched
  dma_gather calls (InstDMAGatherAnt; ~1us SWDGE fixed cost amortized over
  ~5-10k rows) fetch h_full[src] rows; host-precomputed one-hot dst masks
  stream from DRAM; PE matmul (h_g.T @ mask) accumulates in PSUM per dst
  block = transposed segment sum. Conv weight folded AFTER aggregation
  (linearity). GRU runs in transposed [feature, node] layout; PE transposes
  produce the row-major h for the next AllGather / final pooling.
- Mean-pool via host-built batch one-hot matmul + 1/count scale; host sums
  the 8 per-core partials (unshard-reduce).
"""
import contextlib
import os
import sys
import types

import numpy as np

from concourse import bass, mybir, tile
from concourse.bass_utils import run_bass_kernel_spmd

NCORES = 8
P = 128
D = 128
G = 64
N = 50000
V = 100000
NUM_LAYERS = 2
NL = N // NCORES            # 6250 nodes per core
NB = (NL + P - 1) // P      # 49 dst blocks per core
NLP = NB * P                # 6272 padded nodes per core
NFULL = NCORES * NLP        # 50176 rows in allgathered h
LOWN = 32768                # int16 index range split point
GB = 1                      # dst blocks per gather group (SWDGE ring capacity
                            # bounds per-call descriptor count)

_F32 = mybir.dt.float32
_BF16 = mybir.dt.bfloat16
_I32 = mybir.dt.int32
_I16 = mybir.dt.int16


# ---------------------------------------------------------------- wait split
def _split_waits(nc):
    """walrus allows only ONE sync-wait per instruction; hoist extras onto
    NoOps just before, on the same engine stream (sequencer order)."""
    uid = 0
    n_fixed = 0
    for bb in nc.main_func.blocks:
        out = []
        for ins in bb.instructions:
            si = getattr(ins, "sync_info", None)
            if si is not None and len(si.on_wait) > 1:
                for w in si.on_wait[:-1]:
                    uid += 1
                    out.append(mybir.InstNoOp(
                        name=f"WSPLIT-{uid}", engine=ins.engine,
                        bass_nofuse=True, ins=[], outs=[],
                        sync_info=mybir.SyncInfo(on_wait=[w], on_update=[]),
                    ))
                ins.sync_info = mybir.SyncInfo(
                    on_wait=[si.on_wait[-1]], on_update=si.on_update)
                n_fixed += 1
            out.append(ins)
        bb.instructions = out
    return n_fixed


# ------------------------------------------------------------- library loads
def _insert_library_loads(nc):
    """Bacc compile steps the plain-Bass path skips: insert GPSIMD library
    reloads for custom instructions (dma_gather needs the mlp library) and
    encode the resulting pseudo instructions into real MPC ISA structs."""
    import bass_rust as _bass_rust
    from concourse.library_config import all_libraries, standard
    inst_type_to_lib_mask = {}
    for lib in all_libraries:
        for inst_type in lib.instructions:
            inst_type_to_lib_mask[inst_type] = inst_type_to_lib_mask.get(
                inst_type, 0) | (1 << lib.index)
    _bass_rust.insert_library_loads(
        nc, inst_type_to_lib_mask, len(all_libraries), standard.index)
    mybir.codegen_inst_isa_subclasses(nc)


# ---------------------------------------------------------------- ntff hook
def _install_ntff_hook():
    import antenv
    if "antenv.axon_hooks" in sys.modules:
        return
    mod = types.ModuleType("antenv.axon_hooks")
    _state = {"hook": None}
    mod.set_axon_ntff_profile_hook = lambda h: _state.__setitem__("hook", h)
    mod.get_axon_ntff_profile_hook = lambda: _state["hook"]
    sys.modules["antenv.axon_hooks"] = mod
    antenv.axon_hooks = mod
    if "/root/.axon_site" not in sys.path:
        sys.path.insert(0, "/root/.axon_site")
    try:
        from trn_agent_boot.trn_boot import _ntff_profile_via_ctypes
        hook = _ntff_profile_via_ctypes("/opt/axon/libaxon_pjrt.so")
        mod.set_axon_ntff_profile_hook(hook)
    except Exception:
        pass


def _groups():
    """Block groups for gather batching: sizes [GB]*q + [r]."""
    out = []
    j0 = 0
    while j0 < NB:
        gb = min(GB, NB - j0)
        out.append((j0, gb))
        j0 += gb
    return out


# ---------------------------------------------------------------- builder
MAXC = 8   # max tiles (128 idxs each) per dma_gather call: the fixed
           # 1024-descriptor SWDGE ring hangs on larger calls (HW-probed)


NQ = 4     # SWDGE queues; rotating gathers across them lets the 4 Q7
           # contexts overlap the (blocking) transfer phases


def _build(capl: int, caph: int, phases: int = 99):
    nc = bass.Bass(num_devices=NCORES, num_swdge_queues=NQ)
    tpb = capl + caph           # tiles per dst block
    T = NB * tpb                # mask tiles per core per layer
    groups = _groups()
    # idx16 column offsets per (group, range)
    colw_l = [gb * capl * 8 for _, gb in groups]
    colw_h = [gb * caph * 8 for _, gb in groups]
    cols_total = sum(colw_l) + sum(colw_h)

    embed_in = nc.declare_dram_parameter("embed", [V, D], _F32, isOutput=False)
    nid_in = nc.declare_dram_parameter("nid", [P, NB], _I32, isOutput=False)
    idx_in = nc.declare_dram_parameter("idx16", [P, cols_total], _I16, isOutput=False)
    mask_in = nc.declare_dram_parameter("masks", [P, T * D], _BF16, isOutput=False)
    pool_in = nc.declare_dram_parameter("pool1h", [P, NB * G], _BF16, isOutput=False)
    cinv_in = nc.declare_dram_parameter("cinv", [G, 1], _F32, isOutput=False)
    convw_in = nc.declare_dram_parameter("convw", [D, NUM_LAYERS * D], _F32, isOutput=False)
    wih_in = nc.declare_dram_parameter("wihT", [D, 3 * D], _F32, isOutput=False)
    whh_in = nc.declare_dram_parameter("whhT", [D, 3 * D], _F32, isOutput=False)
    bias_in = nc.declare_dram_parameter("biases", [P, 4], _F32, isOutput=False)
    out_ext = nc.declare_dram_parameter("out", [G, D], _F32, isOutput=True)

    ag_in = [nc.dram_tensor(f"ag_in{l}", [NLP, D], _F32) for l in range(NUM_LAYERS)]
    ag_out = [nc.dram_tensor(f"ag_out{l}", [NFULL, D], _F32, addr_space="Shared")
              for l in range(NUM_LAYERS)]

    with tile.TileContext(nc) as tc:
        with contextlib.ExitStack() as stk:
            const = stk.enter_context(tc.tile_pool(name="const", bufs=1))
            sb = stk.enter_context(tc.tile_pool(name="sb", bufs=3))
            gp = stk.enter_context(tc.tile_pool(name="gp", bufs=2))
            pp = stk.enter_context(tc.tile_pool(name="pp", bufs=2, space="PSUM"))
            gpsum = stk.enter_context(tc.tile_pool(name="gpsum", bufs=1, space="PSUM"))

            # ---- constants / weights ----
            idx_sb = const.tile([P, cols_total], _I16)
            nc.sync.dma_start(out=idx_sb[:], in_=idx_in[:])
            nid_sb = const.tile([P, NB], _I32)
            nc.sync.dma_start(out=nid_sb[:], in_=nid_in[:])
            pool_sb = const.tile([P, NB * G], _BF16)
            nc.sync.dma_start(out=pool_sb[:], in_=pool_in[:])
            cinv_sb = const.tile([G, 1], _F32)
            nc.sync.dma_start(out=cinv_sb[:], in_=cinv_in[:])
            bias_sb = const.tile([P, 4], _F32)
            nc.sync.dma_start(out=bias_sb[:], in_=bias_in[:])

            def _load_bf16(src_ap, shape, nm):
                t32 = sb.tile(shape, _F32, name=f"t32_{nm}", tag=f"t32_{nm}")
                nc.sync.dma_start(out=t32[:], in_=src_ap)
                tb = const.tile(shape, _BF16, name=f"bf_{nm}", tag=f"bf_{nm}")
                nc.scalar.copy(out=tb[:], in_=t32[:])
                return tb

            convw_sb = _load_bf16(convw_in[:], [D, NUM_LAYERS * D], "convw")
            wih_sb = _load_bf16(wih_in[:], [D, 3 * D], "wih")
            whh_sb = _load_bf16(whh_in[:], [D, 3 * D], "whh")

            from concourse.masks import make_identity
            ident = const.tile([P, P], _BF16)
            make_identity(nc, ident[:])

            # ---- persistent state buffers ----
            hT = [const.tile([P, NLP], _BF16, name=f"hT{i}", tag=f"hT{i}") for i in range(2)]
            hnorm = const.tile([P, NLP], _BF16)   # row-major h: block b at cols [b*D,(b+1)*D)
            aggT = const.tile([P, NLP], _BF16)

            # ---- phase 1: embed gather -> hnorm + hT[0] ----
            for b in range(NB):
                g32 = sb.tile([P, D], _F32, tag="embg")
                nc.gpsimd.indirect_dma_start(
                    out=g32[:], out_offset=None, in_=embed_in[:],
                    in_offset=bass.IndirectOffsetOnAxis(ap=nid_sb[:, b:b + 1], axis=0))
                nc.scalar.copy(out=hnorm[:, b * D:(b + 1) * D], in_=g32[:])
                tp = pp.tile([P, P], _BF16, tag="scratch", space="PSUM")
                nc.tensor.transpose(out=tp[:], in_=hnorm[:, b * D:(b + 1) * D], identity=ident[:])
                nc.scalar.copy(out=hT[0][:, b * P:(b + 1) * P], in_=tp[:])
            nc.gpsimd.dma_start(
                out=ag_in[0][:].rearrange("(b p) d -> p b d", p=P),
                in_=hnorm[:].rearrange("p (b d) -> p b d", d=D))

            # num_idxs register per distinct value (to_reg per call exhausts
            # the gpsimd register pool)
            nidx_regs = {}
            for cap in (capl, caph):
                a = 0
                while a < cap:
                    k = min(MAXC, cap - a)
                    if k * P not in nidx_regs:
                        nidx_regs[k * P] = nc.gpsimd.to_reg(k * P)
                    a += k

            # ---- layers ----
            for l in range(NUM_LAYERS if phases >= 2 else 0):
                nc.gpsimd.collective_compute(
                    "AllGather", mybir.AluOpType.bypass,
                    replica_groups=[list(range(NCORES))],
                    ins=[ag_in[l][:]], outs=[ag_out[l][:]])

                # edge phase: batched row gathers + per-block mask matmuls
                colp = 0
                qctr = [0]
                for gi, (j0, gb) in enumerate(groups):
                    g32 = gp.tile([P, gb * tpb * D], _F32, tag="g32")
                    for base, cap, src_ap in (
                            (0, capl, ag_out[l][0:LOWN, :]),
                            (capl, caph, ag_out[l][LOWN:NFULL, :])):
                        a = 0
                        while a < cap:
                            k = min(MAXC, cap - a)
                            nc.gpsimd.dma_gather(
                                out_ap=g32[:, (base + a) * D:(base + a + k) * D]
                                    .rearrange("p (t d) -> p t d", d=D),
                                in_ap=src_ap,
                                idxs_ap=idx_sb[:, colp + a * 8:colp + (a + k) * 8],
                                num_idxs=k * P, num_idxs_reg=nidx_regs[k * P],
                                elem_size=D, queue_num=qctr[0] % NQ)
                            qctr[0] += 1
                            a += k
                        colp += cap * 8
                    gblk = gp.tile([P, gb * tpb * D], _BF16, tag="gath")
                    if gi % 2 == 0:
                        nc.vector.tensor_copy(out=gblk[:], in_=g32[:])
                    else:
                        nc.scalar.copy(out=gblk[:], in_=g32[:])

                    for j in range(gb):
                        b = j0 + j
                        pagg = pp.tile([P, P], _F32, tag="scratch", space="PSUM")
                        mblk = sb.tile([P, tpb * D], _BF16, tag="mblk")
                        nc.sync.dma_start(
                            out=mblk[:], in_=mask_in[:, b * tpb * D:(b + 1) * tpb * D])
                        for t in range(tpb):
                            if t < capl:
                                chunk = j * capl + t
                            else:
                                chunk = gb * capl + j * caph + (t - capl)
                            nc.tensor.matmul(
                                out=pagg[:], lhsT=gblk[:, chunk * D:(chunk + 1) * D],
                                rhs=mblk[:, t * D:(t + 1) * D],
                                start=(t == 0), stop=(t == tpb - 1))
                        nc.scalar.copy(out=aggT[:, b * P:(b + 1) * P], in_=pagg[:])

                if phases < 3:
                    continue
                # conv + GRU phase, slabs of 512 nodes
                W = 512
                nslab = NLP // W if NLP % W == 0 else NLP // W + 1
                hT_next = hT[(l + 1) % 2]
                for s in range(nslab):
                    c0 = s * W
                    w = min(W, NLP - c0)
                    cs = slice(c0, c0 + w)
                    xt_ps = gpsum.tile([P, W], _F32, tag="gi0", space="PSUM")
                    nc.tensor.matmul(out=xt_ps[:, :w], lhsT=convw_sb[:, l * D:(l + 1) * D],
                                     rhs=aggT[:, cs], start=True, stop=True)
                    xt_sb = sb.tile([P, W], _BF16, tag="xtsb")
                    nc.scalar.copy(out=xt_sb[:, :w], in_=xt_ps[:, :w])

                    gi = []
                    gh = []
                    for gidx in range(3):
                        gps = gpsum.tile([P, W], _F32, tag=f"gi{gidx}", space="PSUM")
                        nc.tensor.matmul(out=gps[:, :w], lhsT=wih_sb[:, gidx * D:(gidx + 1) * D],
                                         rhs=xt_sb[:, :w], start=True, stop=True)
                        gi.append(gps)
                        hps = gpsum.tile([P, W], _F32, tag=f"gh{gidx}", space="PSUM")
                        nc.tensor.matmul(out=hps[:, :w], lhsT=whh_sb[:, gidx * D:(gidx + 1) * D],
                                         rhs=hT[l % 2][:, cs], start=True, stop=True)
                        gh.append(hps)

                    # r = sigmoid(gi_r + gh_r + b_r) ; z likewise
                    r_sb = sb.tile([P, W], _F32, tag="r")
                    nc.scalar.activation(out=r_sb[:, :w], in_=gh[0][:, :w],
                                         func=mybir.ActivationFunctionType.Identity,
                                         bias=bias_sb[:, 0:1])
                    nc.vector.tensor_tensor(out=r_sb[:, :w], in0=gi[0][:, :w], in1=r_sb[:, :w],
                                            op=mybir.AluOpType.add)
                    nc.scalar.activation(out=r_sb[:, :w], in_=r_sb[:, :w],
                                         func=mybir.ActivationFunctionType.Sigmoid)
                    z_sb = sb.tile([P, W], _F32, tag="z")
                    nc.scalar.activation(out=z_sb[:, :w], in_=gh[1][:, :w],
                                         func=mybir.ActivationFunctionType.Identity,
                                         bias=bias_sb[:, 1:2])
                    nc.vector.tensor_tensor(out=z_sb[:, :w], in0=gi[1][:, :w], in1=z_sb[:, :w],
                                            op=mybir.AluOpType.add)
                    nc.scalar.activation(out=z_sb[:, :w], in_=z_sb[:, :w],
                                         func=mybir.ActivationFunctionType.Sigmoid)
                    # n = tanh(gi_n + b_in + r * (gh_n + b_hn))
                    hn_sb = sb.tile([P, W], _F32, tag="hn")
                    nc.scalar.activation(out=hn_sb[:, :w], in_=gh[2][:, :w],
                                         func=mybir.ActivationFunctionType.Identity,
                                         bias=bias_sb[:, 3:4])
                    nc.vector.tensor_tensor(out=hn_sb[:, :w], in0=r_sb[:, :w], in1=hn_sb[:, :w],
                                            op=mybir.AluOpType.mult)
                    nc.vector.tensor_tensor(out=hn_sb[:, :w], in0=hn_sb[:, :w], in1=gi[2][:, :w],
                                            op=mybir.AluOpType.add)
                    nc.scalar.activation(out=hn_sb[:, :w], in_=hn_sb[:, :w],
                                         func=mybir.ActivationFunctionType.Tanh,
                                         bias=bias_sb[:, 2:3])
                    # h' = n + z*(h - n)
                    d_sb = sb.tile([P, W], _F32, tag="d")
                    nc.vector.tensor_tensor(out=d_sb[:, :w], in0=hT[l % 2][:, cs], in1=hn_sb[:, :w],
                                            op=mybir.AluOpType.subtract)
                    nc.vector.tensor_tensor(out=d_sb[:, :w], in0=z_sb[:, :w], in1=d_sb[:, :w],
                                            op=mybir.AluOpType.mult)
                    nc.vector.tensor_tensor(out=hT_next[:, cs], in0=d_sb[:, :w], in1=hn_sb[:, :w],
                                            op=mybir.AluOpType.add)

                # transpose h'T back to row-major hnorm
                for b in range(NB):
                    tp = pp.tile([P, P], _BF16, tag="scratch", space="PSUM")
                    nc.tensor.transpose(out=tp[:], in_=hT_next[:, b * P:(b + 1) * P],
                                        identity=ident[:])
                    nc.scalar.copy(out=hnorm[:, b * D:(b + 1) * D], in_=tp[:])
                if l + 1 < NUM_LAYERS:
                    nc.gpsimd.dma_start(
                        out=ag_in[l + 1][:].rearrange("(b p) d -> p b d", p=P),
                        in_=hnorm[:].rearrange("p (b d) -> p b d", d=D))

            # ---- pool ----
            if phases < 4:
                out_sb0 = sb.tile([G, D], _F32, tag="outsb")
                nc.vector.memset(out_sb0[:], 0.0)
                nc.sync.dma_start(out=out_ext[:], in_=out_sb0[:])
            else:
                ppool = pp.tile([G, D], _F32, tag="scratch", space="PSUM")
                for b in range(NB):
                    nc.tensor.matmul(out=ppool[:], lhsT=pool_sb[:, b * G:(b + 1) * G],
                                     rhs=hnorm[:, b * D:(b + 1) * D],
                                     start=(b == 0), stop=(b == NB - 1))
                out_sb = sb.tile([G, D], _F32, tag="outsb")
                nc.vector.tensor_scalar(out=out_sb[:], in0=ppool[:], scalar1=cinv_sb[:, 0:1],
                                        scalar2=None, op0=mybir.AluOpType.mult)
                nc.sync.dma_start(out=out_ext[:], in_=out_sb[:])

    _split_waits(nc)
    _insert_library_loads(nc)
    return nc


_CACHE = {}


def kernel(node_ids, edge_index, batch, num_graphs, embed, conv_w, w_ih, w_hh,
           b_ih, b_hh) -> np.ndarray:
    import ml_dtypes
    bf16 = ml_dtypes.bfloat16

    node_ids = np.asarray(node_ids)
    edge_index = np.asarray(edge_index)
    batch = np.asarray(batch)
    embed = np.asarray(embed, dtype=np.float32)
    conv_w = np.asarray(conv_w, dtype=np.float32)
    w_ih = np.asarray(w_ih, dtype=np.float32)
    w_hh = np.asarray(w_hh, dtype=np.float32)
    b_ih = np.asarray(b_ih, dtype=np.float32)
    b_hh = np.asarray(b_hh, dtype=np.float32)
    G_ = int(num_graphs)
    assert G_ == G and node_ids.shape[0] == N

    src_all = edge_index[0].astype(np.int64)
    dst_all = edge_index[1].astype(np.int64)

    # global padded row index of node n in ag_out
    def padded_idx(n):
        return (n // NL) * NLP + (n % NL)

    # ---- pass 1: per-core per-block low/high edge lists; find capacities ----
    owner = dst_all // NL
    per_core = []
    capl = caph = 1
    for c in range(NCORES):
        sel = owner == c
        src_c = padded_idx(src_all[sel]).astype(np.int64)
        dst_c = dst_all[sel] - c * NL          # 0..NL-1
        blk = dst_c // P
        rel = dst_c % P
        lows = []
        highs = []
        for b in range(NB):
            bs = blk == b
            s_b, r_b = src_c[bs], rel[bs]
            lo = s_b < LOWN
            lows.append((s_b[lo], r_b[lo]))
            highs.append((s_b[~lo] - LOWN, r_b[~lo]))
            capl = max(capl, -(-lows[-1][0].shape[0] // P))
            caph = max(caph, -(-highs[-1][0].shape[0] // P))
        per_core.append((lows, highs))
    tpb = capl + caph
    T = NB * tpb
    groups = _groups()

    # ---- common tensors ----
    convw_arr = np.ascontiguousarray(np.concatenate([conv_w[i] for i in range(NUM_LAYERS)], axis=1))
    wihT = np.ascontiguousarray(w_ih.T)           # [128, 384]
    whhT = np.ascontiguousarray(w_hh.T)
    biases = np.zeros((P, 4), np.float32)
    biases[:, 0] = b_ih[0:D] + b_hh[0:D]          # r
    biases[:, 1] = b_ih[D:2 * D] + b_hh[D:2 * D]  # z
    biases[:, 2] = b_ih[2 * D:3 * D]              # in
    biases[:, 3] = b_hh[2 * D:3 * D]              # hn
    counts_g = np.bincount(batch, minlength=G).astype(np.float32)
    cinv = (1.0 / np.maximum(counts_g, 1.0)).reshape(G, 1).astype(np.float32)

    # idx16 columns per group/range
    cols_total = sum(gb * capl * 8 + gb * caph * 8 for _, gb in groups)

    in_maps = []
    for c in range(NCORES):
        lows, highs = per_core[c]
        masks = np.zeros((P, T * D), dtype=bf16)
        idx16 = np.zeros((16, cols_total), dtype=np.int16)
        colp = 0
        for j0, gb in groups:
            for rng, cap, base in ((0, capl, 0), (1, caph, capl)):
                nidx = gb * cap * P
                seq = np.zeros(nidx, np.int16)
                for j in range(gb):
                    b = j0 + j
                    s_b, r_b = (lows[b] if rng == 0 else highs[b])
                    k = s_b.shape[0]
                    seq[j * cap * P:j * cap * P + k] = s_b.astype(np.int16)
                    # mask entries: edge i -> tile i//P, slot i%P
                    ar = np.arange(k)
                    tt = b * tpb + base + ar // P
                    masks[ar % P, tt * D + r_b] = np.float32(1.0)
                idx16[:, colp:colp + nidx // 16] = seq.reshape(-1, 16).T
                colp += nidx // 16
        idx16_full = np.tile(idx16, (8, 1))       # replicate across Q7 cores

        # node ids per padded slot, [128, NB] column-major tiles
        nid = np.zeros((P, NB), np.int32)
        ids_c = node_ids[c * NL:(c + 1) * NL].astype(np.int32)
        ids_pad = np.zeros(NLP, np.int32)
        ids_pad[:NL] = ids_c
        nid[:, :] = ids_pad.reshape(NB, P).T
        # pool one-hot [128, NB*G]
        b_c = batch[c * NL:(c + 1) * NL].astype(np.int64)
        p1h = np.zeros((NLP, G), dtype=bf16)
        p1h[np.arange(NL), b_c] = np.float32(1.0)
        pool1h = np.zeros((P, NB * G), dtype=bf16)
        for b in range(NB):
            pool1h[:, b * G:(b + 1) * G] = p1h[b * P:(b + 1) * P, :]

        in_maps.append({
            "embed": embed, "nid": nid, "idx16": idx16_full, "masks": masks,
            "pool1h": pool1h, "cinv": cinv, "convw": convw_arr,
            "wihT": wihT, "whhT": whhT, "biases": biases,
        })

    key = (capl, caph)
    if key not in _CACHE:
        _CACHE[key] = _build(capl, caph)
    nc = _CACHE[key]

    if bool(int(os.environ.get("BASS_GNN_SIM", "0"))):
        from concourse.bass_interp import MultiCoreSim
        sim = MultiCoreSim(nc, num_cores=NCORES)
        for c in range(NCORES):
            for nm, arr in in_maps[c].items():
                sim.cores[c].tensor(nm)[:] = arr
        sim.simulate(check_with_hw=False)
        outs = [np.array(sim.cores[c].tensor("out")) for c in range(NCORES)]
        return np.sum(np.stack(outs, 0), axis=0, dtype=np.float32)

    trace = bool(int(os.environ.get("BASS_GNN_TRACE", "0")))
    if trace:
        _install_ntff_hook()
    res = run_bass_kernel_spmd(nc, in_maps, core_ids=list(range(NCORES)),
                               trace=trace)
    if trace:
        kernel.last_exec_time_ns = res.exec_time_ns
        kernel.last_results = res
    outs = [r["out"] for r in res.results]
    return np.sum(np.stack(outs, 0), axis=0, dtype=np.float32)


kernel.last_exec_time_ns = None


# revision 20
# speedup vs baseline: 1.6664x; 1.0798x over previous
"""GatedConv GNN message passing on 8 TRN2 NeuronCores.

Strategy:
- Nodes sharded contiguously across 8 cores (6250/core, padded to 6272=49*128).
- Edges sharded by dst owner, sorted by dst, grouped into 128-node dst blocks.
  Per (core, block) the edges are split into "low" (padded src idx < 32768)
  and "high" ranges so indices fit int16, padded to uniform per-block tile
  capacities CAPL/CAPH so one SPMD program serves all cores.
- Per layer: AllGather h (bf16) -> per group of GB dst blocks: two batched
  dma_gather calls (InstDMAGatherAnt; ~1us SWDGE fixed cost amortized over
  ~5-10k rows) fetch h_full[src] rows; host-precomputed one-hot dst masks
  stream from DRAM; PE matmul (h_g.T @ mask) accumulates in PSUM per dst
  block = transposed segment sum. Conv weight folded AFTER aggregation
  (linearity). GRU runs in transposed [feature, node] layout; PE transposes
  produce the row-major h for the next AllGather / final pooling.
- Mean-pool via host-built batch one-hot matmul + 1/count scale; host sums
  the 8 per-core partials (unshard-reduce).
"""
import contextlib
import os
import sys
import types

import numpy as np

from concourse import bass, mybir, tile
from concourse.bass_utils import run_bass_kernel_spmd

NCORES = 8
P = 128
D = 128
G = 64
N = 50000
V = 100000
NUM_LAYERS = 2
NL = N // NCORES            # 6250 nodes per core
NB = (NL + P - 1) // P      # 49 dst blocks per core
NLP = NB * P                # 6272 padded nodes per core
NFULL = NCORES * NLP        # 50176 rows in allgathered h
NR = 4                      # index ranges (doubled indices / int16 limit)
RNG = 16384                 # original-index width per range
GB = 1                      # dst blocks per gather group (SWDGE ring capacity
                            # bounds per-call descriptor count)

_F32 = mybir.dt.float32
_BF16 = mybir.dt.bfloat16
_I32 = mybir.dt.int32
_I16 = mybir.dt.int16


# ---------------------------------------------------------------- wait split
def _split_waits(nc):
    """walrus allows only ONE sync-wait per instruction; hoist extras onto
    NoOps just before, on the same engine stream (sequencer order)."""
    uid = 0
    n_fixed = 0
    for bb in nc.main_func.blocks:
        out = []
        for ins in bb.instructions:
            si = getattr(ins, "sync_info", None)
            if si is not None and len(si.on_wait) > 1:
                for w in si.on_wait[:-1]:
                    uid += 1
                    out.append(mybir.InstNoOp(
                        name=f"WSPLIT-{uid}", engine=ins.engine,
                        bass_nofuse=True, ins=[], outs=[],
                        sync_info=mybir.SyncInfo(on_wait=[w], on_update=[]),
                    ))
                ins.sync_info = mybir.SyncInfo(
                    on_wait=[si.on_wait[-1]], on_update=si.on_update)
                n_fixed += 1
            out.append(ins)
        bb.instructions = out
    return n_fixed


# ------------------------------------------------------------- library loads
def _insert_library_loads(nc):
    """Bacc compile steps the plain-Bass path skips: insert GPSIMD library
    reloads for custom instructions (dma_gather needs the mlp library) and
    encode the resulting pseudo instructions into real MPC ISA structs."""
    import bass_rust as _bass_rust
    from concourse.library_config import all_libraries, standard
    inst_type_to_lib_mask = {}
    for lib in all_libraries:
        for inst_type in lib.instructions:
            inst_type_to_lib_mask[inst_type] = inst_type_to_lib_mask.get(
                inst_type, 0) | (1 << lib.index)
    _bass_rust.insert_library_loads(
        nc, inst_type_to_lib_mask, len(all_libraries), standard.index)
    mybir.codegen_inst_isa_subclasses(nc)


# ---------------------------------------------------------------- ntff hook
def _install_ntff_hook():
    import antenv
    if "antenv.axon_hooks" in sys.modules:
        return
    mod = types.ModuleType("antenv.axon_hooks")
    _state = {"hook": None}
    mod.set_axon_ntff_profile_hook = lambda h: _state.__setitem__("hook", h)
    mod.get_axon_ntff_profile_hook = lambda: _state["hook"]
    sys.modules["antenv.axon_hooks"] = mod
    antenv.axon_hooks = mod
    if "/root/.axon_site" not in sys.path:
        sys.path.insert(0, "/root/.axon_site")
    try:
        from trn_agent_boot.trn_boot import _ntff_profile_via_ctypes
        hook = _ntff_profile_via_ctypes("/opt/axon/libaxon_pjrt.so")
        mod.set_axon_ntff_profile_hook(hook)
    except Exception:
        pass


def _groups():
    """Block groups for gather batching: sizes [GB]*q + [r]."""
    out = []
    j0 = 0
    while j0 < NB:
        gb = min(GB, NB - j0)
        out.append((j0, gb))
        j0 += gb
    return out


# ---------------------------------------------------------------- builder
MAXC = 8   # max tiles (128 idxs each) per dma_gather call: the fixed
           # 1024-descriptor SWDGE ring hangs on larger calls (HW-probed)


NQ = 4     # SWDGE queues; rotating gathers across them lets the 4 Q7
           # contexts overlap the (blocking) transfer phases


def _build(caps: tuple, phases: int = 99):
    nc = bass.Bass(num_devices=NCORES, num_swdge_queues=NQ)
    tpb = sum(caps)             # tiles per dst block
    T = NB * tpb                # mask tiles per core per layer
    groups = _groups()
    cols_total = NB * tpb * 8

    embed_in = nc.declare_dram_parameter("embed", [V, D], _F32, isOutput=False)
    nid_in = nc.declare_dram_parameter("nid", [P, NB], _I32, isOutput=False)
    idx_in = nc.declare_dram_parameter("idx16", [P, cols_total], _I16, isOutput=False)
    mask_in = nc.declare_dram_parameter("masks", [P, T * D], _BF16, isOutput=False)
    pool_in = nc.declare_dram_parameter("pool1h", [P, NB * G], _BF16, isOutput=False)
    cinv_in = nc.declare_dram_parameter("cinv", [G, 1], _F32, isOutput=False)
    convw_in = nc.declare_dram_parameter("convw", [D, NUM_LAYERS * D], _F32, isOutput=False)
    wih_in = nc.declare_dram_parameter("wihT", [D, 3 * D], _F32, isOutput=False)
    whh_in = nc.declare_dram_parameter("whhT", [D, 3 * D], _F32, isOutput=False)
    bias_in = nc.declare_dram_parameter("biases", [P, 4], _F32, isOutput=False)
    out_ext = nc.declare_dram_parameter("out", [G, D], _F32, isOutput=True)

    # row n holds [h[n], h[n]]: doubled gather indices 2*src are always even
    # (the dma_gather ucode fetches row idx-1 for odd indices at 256B rows)
    ag_in = [nc.dram_tensor(f"ag_in{l}", [NLP, 2 * D], _BF16) for l in range(NUM_LAYERS)]
    ag_out = [nc.dram_tensor(f"ag_out{l}", [NFULL, 2 * D], _BF16, addr_space="Shared")
              for l in range(NUM_LAYERS)]

    with tile.TileContext(nc) as tc:
        with contextlib.ExitStack() as stk:
            const = stk.enter_context(tc.tile_pool(name="const", bufs=1))
            sb = stk.enter_context(tc.tile_pool(name="sb", bufs=3))
            gp = stk.enter_context(tc.tile_pool(name="gp", bufs=2))
            pp = stk.enter_context(tc.tile_pool(name="pp", bufs=2, space="PSUM"))
            gpsum = stk.enter_context(tc.tile_pool(name="gpsum", bufs=1, space="PSUM"))

            # ---- constants / weights ----
            idx_sb = const.tile([P, cols_total], _I16)
            nc.sync.dma_start(out=idx_sb[:], in_=idx_in[:])
            nid_sb = const.tile([P, NB], _I32)
            nc.sync.dma_start(out=nid_sb[:], in_=nid_in[:])
            pool_sb = const.tile([P, NB * G], _BF16)
            nc.sync.dma_start(out=pool_sb[:], in_=pool_in[:])
            cinv_sb = const.tile([G, 1], _F32)
            nc.sync.dma_start(out=cinv_sb[:], in_=cinv_in[:])
            bias_sb = const.tile([P, 4], _F32)
            nc.sync.dma_start(out=bias_sb[:], in_=bias_in[:])

            def _load_bf16(src_ap, shape, nm):
                t32 = sb.tile(shape, _F32, name=f"t32_{nm}", tag=f"t32_{nm}")
                nc.sync.dma_start(out=t32[:], in_=src_ap)
                tb = const.tile(shape, _BF16, name=f"bf_{nm}", tag=f"bf_{nm}")
                nc.scalar.copy(out=tb[:], in_=t32[:])
                return tb

            convw_sb = _load_bf16(convw_in[:], [D, NUM_LAYERS * D], "convw")
            wih_sb = _load_bf16(wih_in[:], [D, 3 * D], "wih")
            whh_sb = _load_bf16(whh_in[:], [D, 3 * D], "whh")

            from concourse.masks import make_identity
            ident = const.tile([P, P], _BF16)
            make_identity(nc, ident[:])

            # ---- persistent state buffers ----
            hT = [const.tile([P, NLP], _BF16, name=f"hT{i}", tag=f"hT{i}") for i in range(2)]
            hnorm = const.tile([P, NLP], _BF16)   # row-major h: block b at cols [b*D,(b+1)*D)
            aggT = const.tile([P, NLP], _BF16)

            # ---- phase 1: embed gather -> hnorm + hT[0] ----
            for b in range(NB):
                g32 = sb.tile([P, D], _F32, tag="embg")
                nc.gpsimd.indirect_dma_start(
                    out=g32[:], out_offset=None, in_=embed_in[:],
                    in_offset=bass.IndirectOffsetOnAxis(ap=nid_sb[:, b:b + 1], axis=0))
                nc.scalar.copy(out=hnorm[:, b * D:(b + 1) * D], in_=g32[:])
                tp = pp.tile([P, P], _BF16, tag="scratch", space="PSUM")
                nc.tensor.transpose(out=tp[:], in_=hnorm[:, b * D:(b + 1) * D], identity=ident[:])
                nc.scalar.copy(out=hT[0][:, b * P:(b + 1) * P], in_=tp[:])
            nc.gpsimd.dma_start(
                out=ag_in[0][:].rearrange("(b p) d -> p b d", p=P)[:, :, 0:D],
                in_=hnorm[:].rearrange("p (b d) -> p b d", d=D))
            nc.gpsimd.dma_start(
                out=ag_in[0][:].rearrange("(b p) d -> p b d", p=P)[:, :, D:2 * D],
                in_=hnorm[:].rearrange("p (b d) -> p b d", d=D))

            # num_idxs register per distinct value (to_reg per call exhausts
            # the gpsimd register pool)
            nidx_regs = {}
            for cap in caps:
                a = 0
                while a < cap:
                    k = min(MAXC, cap - a)
                    if k * P not in nidx_regs:
                        nidx_regs[k * P] = nc.gpsimd.to_reg(k * P)
                    a += k

            # ---- layers ----
            for l in range(NUM_LAYERS if phases >= 2 else 0):
                nc.gpsimd.collective_compute(
                    "AllGather", mybir.AluOpType.bypass,
                    replica_groups=[list(range(NCORES))],
                    ins=[ag_in[l][:]], outs=[ag_out[l][:]])

                # edge phase: batched row gathers + per-block mask matmuls
                flat = ag_out[l][:].rearrange("n (two d) -> (n two) d", d=D)
                colp = 0
                qctr = [0]
                for gi, (j0, gb) in enumerate(groups):
                    gblk = gp.tile([P, gb * tpb * D], _BF16, tag="gath")
                    base = 0
                    for r in range(NR):
                        cap = caps[r]
                        lo = r * 2 * RNG
                        hi = min(2 * NFULL, (r + 1) * 2 * RNG)
                        src_ap = flat[lo:hi, :]
                        a = 0
                        while a < cap:
                            k = min(MAXC, cap - a)
                            nc.gpsimd.dma_gather(
                                out_ap=gblk[:, (base + a) * D:(base + a + k) * D]
                                    .rearrange("p (t d) -> p t d", d=D),
                                in_ap=src_ap,
                                idxs_ap=idx_sb[:, colp + a * 8:colp + (a + k) * 8],
                                num_idxs=k * P, num_idxs_reg=nidx_regs[k * P],
                                elem_size=D, queue_num=qctr[0] % NQ)
                            qctr[0] += 1
                            a += k
                        colp += cap * 8
                        base += cap

                    for j in range(gb):
                        b = j0 + j
                        pagg = pp.tile([P, P], _F32, tag="scratch", space="PSUM")
                        mblk = sb.tile([P, tpb * D], _BF16, tag="mblk")
                        nc.sync.dma_start(
                            out=mblk[:], in_=mask_in[:, b * tpb * D:(b + 1) * tpb * D])
                        for t in range(tpb):
                            chunk = j * tpb + t
                            nc.tensor.matmul(
                                out=pagg[:], lhsT=gblk[:, chunk * D:(chunk + 1) * D],
                                rhs=mblk[:, t * D:(t + 1) * D],
                                start=(t == 0), stop=(t == tpb - 1))
                        nc.scalar.copy(out=aggT[:, b * P:(b + 1) * P], in_=pagg[:])

                if phases < 3:
                    continue
                # conv + GRU phase, slabs of 512 nodes
                W = 512
                nslab = NLP // W if NLP % W == 0 else NLP // W + 1
                hT_next = hT[(l + 1) % 2]
                for s in range(nslab):
                    c0 = s * W
                    w = min(W, NLP - c0)
                    cs = slice(c0, c0 + w)
                    xt_ps = gpsum.tile([P, W], _F32, tag="gi0", space="PSUM")
                    nc.tensor.matmul(out=xt_ps[:, :w], lhsT=convw_sb[:, l * D:(l + 1) * D],
                                     rhs=aggT[:, cs], start=True, stop=True)
                    xt_sb = sb.tile([P, W], _BF16, tag="xtsb")
                    nc.scalar.copy(out=xt_sb[:, :w], in_=xt_ps[:, :w])

                    gi = []
                    gh = []
                    for gidx in range(3):
                        gps = gpsum.tile([P, W], _F32, tag=f"gi{gidx}", space="PSUM")
                        nc.tensor.matmul(out=gps[:, :w], lhsT=wih_sb[:, gidx * D:(gidx + 1) * D],
                                         rhs=xt_sb[:, :w], start=True, stop=True)
                        gi.append(gps)
                        hps = gpsum.tile([P, W], _F32, tag=f"gh{gidx}", space="PSUM")
                        nc.tensor.matmul(out=hps[:, :w], lhsT=whh_sb[:, gidx * D:(gidx + 1) * D],
                                         rhs=hT[l % 2][:, cs], start=True, stop=True)
                        gh.append(hps)

                    # r = sigmoid(gi_r + gh_r + b_r) ; z likewise
                    r_sb = sb.tile([P, W], _F32, tag="r")
                    nc.scalar.activation(out=r_sb[:, :w], in_=gh[0][:, :w],
                                         func=mybir.ActivationFunctionType.Identity,
                                         bias=bias_sb[:, 0:1])
                    nc.vector.tensor_tensor(out=r_sb[:, :w], in0=gi[0][:, :w], in1=r_sb[:, :w],
                                            op=mybir.AluOpType.add)
                    nc.scalar.activation(out=r_sb[:, :w], in_=r_sb[:, :w],
                                         func=mybir.ActivationFunctionType.Sigmoid)
                    z_sb = sb.tile([P, W], _F32, tag="z")
                    nc.scalar.activation(out=z_sb[:, :w], in_=gh[1][:, :w],
                                         func=mybir.ActivationFunctionType.Identity,
                                         bias=bias_sb[:, 1:2])
                    nc.vector.tensor_tensor(out=z_sb[:, :w], in0=gi[1][:, :w], in1=z_sb[:, :w],
                                            op=mybir.AluOpType.add)
                    nc.scalar.activation(out=z_sb[:, :w], in_=z_sb[:, :w],
                                         func=mybir.ActivationFunctionType.Sigmoid)
                    # n = tanh(gi_n + b_in + r * (gh_n + b_hn))
                    hn_sb = sb.tile([P, W], _F32, tag="hn")
                    nc.scalar.activation(out=hn_sb[:, :w], in_=gh[2][:, :w],
                                         func=mybir.ActivationFunctionType.Identity,
                                         bias=bias_sb[:, 3:4])
                    nc.vector.tensor_tensor(out=hn_sb[:, :w], in0=r_sb[:, :w], in1=hn_sb[:, :w],
                                            op=mybir.AluOpType.mult)
                    nc.vector.tensor_tensor(out=hn_sb[:, :w], in0=hn_sb[:, :w], in1=gi[2][:, :w],
                                            op=mybir.AluOpType.add)
                    nc.scalar.activation(out=hn_sb[:, :w], in_=hn_sb[:, :w],
                                         func=mybir.ActivationFunctionType.Tanh,
                                         bias=bias_sb[:, 2:3])
                    # h' = n + z*(h - n)
                    d_sb = sb.tile([P, W], _F32, tag="d")
                    nc.vector.tensor_tensor(out=d_sb[:, :w], in0=hT[l % 2][:, cs], in1=hn_sb[:, :w],
                                            op=mybir.AluOpType.subtract)
                    nc.vector.tensor_tensor(out=d_sb[:, :w], in0=z_sb[:, :w], in1=d_sb[:, :w],
                                            op=mybir.AluOpType.mult)
                    nc.vector.tensor_tensor(out=hT_next[:, cs], in0=d_sb[:, :w], in1=hn_sb[:, :w],
                                            op=mybir.AluOpType.add)

                # transpose h'T back to row-major hnorm
                for b in range(NB):
                    tp = pp.tile([P, P], _BF16, tag="scratch", space="PSUM")
                    nc.tensor.transpose(out=tp[:], in_=hT_next[:, b * P:(b + 1) * P],
                                        identity=ident[:])
                    nc.scalar.copy(out=hnorm[:, b * D:(b + 1) * D], in_=tp[:])
                if l + 1 < NUM_LAYERS:
                    nc.gpsimd.dma_start(
                        out=ag_in[l + 1][:].rearrange("(b p) d -> p b d", p=P)[:, :, 0:D],
                        in_=hnorm[:].rearrange("p (b d) -> p b d", d=D))
                    nc.gpsimd.dma_start(
                        out=ag_in[l + 1][:].rearrange("(b p) d -> p b d", p=P)[:, :, D:2 * D],
                        in_=hnorm[:].rearrange("p (b d) -> p b d", d=D))

            # ---- pool ----
            if phases < 4:
                out_sb0 = sb.tile([G, D], _F32, tag="outsb")
                nc.vector.memset(out_sb0[:], 0.0)
                nc.sync.dma_start(out=out_ext[:], in_=out_sb0[:])
            else:
                ppool = pp.tile([G, D], _F32, tag="scratch", space="PSUM")
                for b in range(NB):
                    nc.tensor.matmul(out=ppool[:], lhsT=pool_sb[:, b * G:(b + 1) * G],
                                     rhs=hnorm[:, b * D:(b + 1) * D],
                                     start=(b == 0), stop=(b == NB - 1))
                out_sb = sb.tile([G, D], _F32, tag="outsb")
                nc.vector.tensor_scalar(out=out_sb[:], in0=ppool[:], scalar1=cinv_sb[:, 0:1],
                                        scalar2=None, op0=mybir.AluOpType.mult)
                nc.sync.dma_start(out=out_ext[:], in_=out_sb[:])

    _split_waits(nc)
    _insert_library_loads(nc)
    return nc


_CACHE = {}


def kernel(node_ids, edge_index, batch, num_graphs, embed, conv_w, w_ih, w_hh,
           b_ih, b_hh) -> np.ndarray:
    import ml_dtypes
    bf16 = ml_dtypes.bfloat16

    node_ids = np.asarray(node_ids)
    edge_index = np.asarray(edge_index)
    batch = np.asarray(batch)
    embed = np.asarray(embed, dtype=np.float32)
    conv_w = np.asarray(conv_w, dtype=np.float32)
    w_ih = np.asarray(w_ih, dtype=np.float32)
    w_hh = np.asarray(w_hh, dtype=np.float32)
    b_ih = np.asarray(b_ih, dtype=np.float32)
    b_hh = np.asarray(b_hh, dtype=np.float32)
    G_ = int(num_graphs)
    assert G_ == G and node_ids.shape[0] == N

    src_all = edge_index[0].astype(np.int64)
    dst_all = edge_index[1].astype(np.int64)

    # global padded row index of node n in ag_out
    def padded_idx(n):
        return (n // NL) * NLP + (n % NL)

    # ---- pass 1: per-core per-block per-range edge lists; find capacities ----
    owner = dst_all // NL
    per_core = []
    caps = [1] * NR
    for c in range(NCORES):
        sel = owner == c
        src_c = padded_idx(src_all[sel]).astype(np.int64)
        dst_c = dst_all[sel] - c * NL          # 0..NL-1
        blk = dst_c // P
        rel = dst_c % P
        rng_of = src_c // RNG
        blists = []
        for b in range(NB):
            bs = blk == b
            s_b, r_b, g_b = src_c[bs], rel[bs], rng_of[bs]
            rlists = []
            for r in range(NR):
                rs = g_b == r
                # doubled index within range: always even, fits int16
                rlists.append((2 * (s_b[rs] - r * RNG), r_b[rs]))
                caps[r] = max(caps[r], -(-rlists[-1][0].shape[0] // P))
            blists.append(rlists)
        per_core.append(blists)
    caps = tuple(caps)
    tpb = sum(caps)
    T = NB * tpb
    groups = _groups()

    # ---- common tensors ----
    convw_arr = np.ascontiguousarray(np.concatenate([conv_w[i] for i in range(NUM_LAYERS)], axis=1))
    wihT = np.ascontiguousarray(w_ih.T)           # [128, 384]
    whhT = np.ascontiguousarray(w_hh.T)
    biases = np.zeros((P, 4), np.float32)
    biases[:, 0] = b_ih[0:D] + b_hh[0:D]          # r
    biases[:, 1] = b_ih[D:2 * D] + b_hh[D:2 * D]  # z
    biases[:, 2] = b_ih[2 * D:3 * D]              # in
    biases[:, 3] = b_hh[2 * D:3 * D]              # hn
    counts_g = np.bincount(batch, minlength=G).astype(np.float32)
    cinv = (1.0 / np.maximum(counts_g, 1.0)).reshape(G, 1).astype(np.float32)

    cols_total = NB * tpb * 8

    in_maps = []
    for c in range(NCORES):
        blists = per_core[c]
        masks = np.zeros((P, T * D), dtype=bf16)
        idx16 = np.zeros((16, cols_total), dtype=np.int16)
        colp = 0
        for j0, gb in groups:
            for j in range(gb):
                b = j0 + j
                base = 0
                for r in range(NR):
                    cap = caps[r]
                    s_b, r_b = blists[b][r]
                    k = s_b.shape[0]
                    nidx = cap * P
                    seq = np.zeros(nidx, np.int16)
                    seq[:k] = s_b.astype(np.int16)
                    # mask entries: edge i -> tile i//P, slot i%P
                    ar = np.arange(k)
                    tt = b * tpb + base + ar // P
                    masks[ar % P, tt * D + r_b] = np.float32(1.0)
                    idx16[:, colp:colp + nidx // 16] = seq.reshape(-1, 16).T
                    colp += nidx // 16
                    base += cap
        idx16_full = np.tile(idx16, (8, 1))       # replicate across Q7 cores

        # node ids per padded slot, [128, NB] column-major tiles
        nid = np.zeros((P, NB), np.int32)
        ids_c = node_ids[c * NL:(c + 1) * NL].astype(np.int32)
        ids_pad = np.zeros(NLP, np.int32)
        ids_pad[:NL] = ids_c
        nid[:, :] = ids_pad.reshape(NB, P).T
        # pool one-hot [128, NB*G]
        b_c = batch[c * NL:(c + 1) * NL].astype(np.int64)
        p1h = np.zeros((NLP, G), dtype=bf16)
        p1h[np.arange(NL), b_c] = np.float32(1.0)
        pool1h = np.zeros((P, NB * G), dtype=bf16)
        for b in range(NB):
            pool1h[:, b * G:(b + 1) * G] = p1h[b * P:(b + 1) * P, :]

        in_maps.append({
            "embed": embed, "nid": nid, "idx16": idx16_full, "masks": masks,
            "pool1h": pool1h, "cinv": cinv, "convw": convw_arr,
            "wihT": wihT, "whhT": whhT, "biases": biases,
        })

    if caps not in _CACHE:
        _CACHE[caps] = _build(caps)
    nc = _CACHE[caps]

    if bool(int(os.environ.get("BASS_GNN_SIM", "0"))):
        from concourse.bass_interp import MultiCoreSim
        sim = MultiCoreSim(nc, num_cores=NCORES)
        for c in range(NCORES):
            for nm, arr in in_maps[c].items():
                sim.cores[c].tensor(nm)[:] = arr
        sim.simulate(check_with_hw=False)
        outs = [np.array(sim.cores[c].tensor("out")) for c in range(NCORES)]
        return np.sum(np.stack(outs, 0), axis=0, dtype=np.float32)

    trace = bool(int(os.environ.get("BASS_GNN_TRACE", "0")))
    if trace:
        _install_ntff_hook()
    res = run_bass_kernel_spmd(nc, in_maps, core_ids=list(range(NCORES)),
                               trace=trace)
    if trace:
        kernel.last_exec_time_ns = res.exec_time_ns
        kernel.last_results = res
    outs = [r["out"] for r in res.results]
    return np.sum(np.stack(outs, 0), axis=0, dtype=np.float32)


kernel.last_exec_time_ns = None
